# revision 24
# baseline (speedup 1.0000x reference)
"""Sliding-window multi-head attention for Trainium2, 8-core SPMD.

Sharding: sequence-parallel. B=2 batches x 4 chunks of 512 queries = 8 cores.
Each core computes QKV projections for its chunk (+128-row halo for K/V),
banded attention (window 256 -> band |j-s|<=128), and the output projection
for its 512 rows. No collectives; host concatenates the 8 output chunks.

v2: full bf16 pipeline (weights/activations bf16, fp32 PSUM accum).
 - bf16 matmuls: 1 cyc/row at any width (fp32r paid 4x below N=256), FWL
   halves LDWEIGHTS, and DMA bytes halve.
 - QK scores^T are written in three 512-col PSUM chunks per (pair, half) so
   exp runs as 3 big ACT ops instead of 6 small ones.
 - Engine rebalance around "GpSimd cannot read PSUM": all PSUM evictions on
   DVE/ACT, mask-muls mostly on GpSimd (SBUF-only), some on DVE (bf16 2x).
 - Denominators: free via ones-column in AV; recip = exp(-ln(den)) on ACT;
   broadcast to 64 partitions via one K=2 matmul per pair (sel matrix).
 - O-projection emitted transposed (out^T[e,q]) so the bias add is a
   per-partition ACT Identity+bias; host transposes back.
 - Projection work is interleaved with attention so ACT starts exps ~6us in.
"""

import numpy as np
import ml_dtypes

import concourse.bass as bass
import concourse.tile as tile
from concourse import mybir
from concourse.alu_op_type import AluOpType
from concourse.vector_clock import ScopedClock
from concourse.bass_utils import run_bass_kernel_spmd

FP32 = mybir.dt.float32
BF16 = mybir.dt.bfloat16
U32 = mybir.dt.uint32
AF = mybir.ActivationFunctionType
BF = ml_dtypes.bfloat16

# Problem constants (hardcoded per contract)
B, S, IN_DIM, E = 2, 2048, 512, 512
H, HD = 8, 64
WS, HW = 256, 128
CH = 512          # own queries per core
LK = 768          # local keys per core (chunk + 128 halo each side)
NT = 6            # key tiles of 128
W_T = [128, 256, 384, 384, 256, 128]   # valid query-span width per key tile
QS_T = [0, 0, 0, 128, 256, 384]        # local query start per key tile
OFF_T = [0, 128, 384, 768, 1152, 1408]  # column offset in the concat layout
WSUM = 1536

# QK matmuls per (pair, half), grouped by 512-col PSUM chunk:
# (tile, query_start, width, dst_col_in_chunk)
CHUNK_MMS = [
    [(0, 0, 128, 0), (1, 0, 256, 128), (2, 0, 128, 384)],
    [(2, 128, 256, 0), (3, 128, 256, 256)],
    [(3, 384, 128, 0), (4, 256, 256, 128), (5, 384, 128, 384)],
]

_MAX_WAITS = 1
_patched = False


def _split_sync_waits(nc):
    """This container's walrus accepts only 1 sync-wait per instruction.
    Move extra waits onto nofuse NOPs inserted just before, on the same
    engine sequencer (in-order execution makes this equivalent)."""
    n_split = 0
    for fn in nc.m.functions:
        for bb in fn.blocks:
            insts = list(bb.instructions)
            out = []
            for inst in insts:
                si = inst.sync_info
                if si is not None and len(si.on_wait) > _MAX_WAITS:
                    waits = list(si.on_wait)
                    extra, keep = waits[:-_MAX_WAITS], waits[-_MAX_WAITS:]
                    for j in range(0, len(extra), _MAX_WAITS):
                        out.append(
                            mybir.InstNoOp(
                                name=f"{inst.name}-sw{j}",
                                engine=inst.engine,
                                bass_nofuse=True,
                                sync_info=mybir.SyncInfo(
                                    on_wait=extra[j : j + _MAX_WAITS], on_update=[]
                                ),
                            )
                        )
                    inst.sync_info = mybir.SyncInfo(
                        on_wait=keep, on_update=list(si.on_update)
                    )
                    n_split += 1
                out.append(inst)
            if len(out) != len(insts):
                try:
                    bb.instructions = out
                except Exception:
                    bb.instructions[:] = out
    return n_split


def _patch_tile_drain():
    global _patched
    if _patched:
        return
    _patched = True

    def _drain_and_barrier(self, tick_clock, wait_clock):
        nc = self.nc
        drain_inst = nc.sync.drain()
        wait_clock.add_sem_waits(
            drain_inst.ins, ScopedClock({None: tick_clock.global_clock})
        )
        nc.all_engine_barrier()
        assert self.sems is not None
        popped = nc._tile_sem_poison_stack.pop()
        assert popped is self._sem_poison
        nc.clear_and_free_semaphores(list(self.sems.allocated().values()))
        nc.all_engine_barrier()
        _split_sync_waits(nc)

    tile.TileContext._drain_and_barrier = _drain_and_barrier


def _build_program():
    _patch_tile_drain()
    nc = bass.Bass("TRN2", target_bir_lowering=False, debug=False)

    xt = nc.dram_tensor("xt", [IN_DIM, LK], BF16, kind="ExternalInput")
    wq = nc.dram_tensor("wq", [IN_DIM, E], BF16, kind="ExternalInput")
    wk = nc.dram_tensor("wk", [IN_DIM, E], BF16, kind="ExternalInput")
    wv = nc.dram_tensor("wv", [IN_DIM, E], BF16, kind="ExternalInput")
    ow = nc.dram_tensor("ow", [E, E], BF16, kind="ExternalInput")
    mk = nc.dram_tensor("mk", [128, WSUM], BF16, kind="ExternalInput")
    # bigb fp32 [128, 524]: cols 0:4 qb per pair, 4:8 kb, 8:12 obT, 12:524 vb
    bigb = nc.dram_tensor("bigb", [128, 524], FP32, kind="ExternalInput")
    out = nc.dram_tensor("out", [E, CH], BF16, kind="ExternalOutput")

    with tile.TileContext(nc) as tc:
        with (
            tc.tile_pool(name="const", bufs=1) as cpool,
            tc.tile_pool(name="proj", bufs=1) as ppool,
            tc.tile_pool(name="att", bufs=4) as apool,
            tc.tile_pool(name="small", bufs=2) as spool,
            tc.tile_pool(name="fin", bufs=2) as fpool,
            tc.tile_pool(name="ps", bufs=4, space="PSUM") as ps,
        ):
            # ---- input DMAs ----
            # The SDMA engines round-robin between all in-flight DMAs at
            # packet granularity, so everything dispatched together finishes
            # together. To get xt+wq (the q-projection inputs) early, gate
            # the remaining transfers behind tiny SBUF->SBUF reads of the
            # freshly landed tiles (the gate blocks its queue's dispatch
            # until the previous transfer completes).
            bb_t = cpool.tile([128, 524], FP32, tag="bigb", name="bb_t")
            nc.sync.dma_start(bb_t[:], bigb[:])
            xt_t = cpool.tile([128, 4, LK], BF16, tag="xt", name="xt_t")
            nc.sync.dma_start(xt_t[:], xt[:].rearrange("(c p) j -> p c j", p=128))
            wq_t = cpool.tile([128, 4, E], BF16, tag="wq", name="wq_t")
            nc.scalar.dma_start(wq_t[:], wq[:].rearrange("(c p) e -> p c e", p=128))
            # Deferred tiles get a tiny "gate" DMA writing their last corner,
            # sourced from the prioritized tile. The WAW overlap forces the
            # real transfer to wait until xt/wq fully land (a plain queue-
            # order gate fails: the sequencer parks waiting DMAs in a 4-deep
            # side queue and keeps dispatching later ones).
            wk_t = cpool.tile([128, 4, E], BF16, tag="wk", name="wk_t")
            wv_t = cpool.tile([128, 4, E], BF16, tag="wv", name="wv_t")
            mk_t = cpool.tile([128, WSUM], BF16, tag="mk", name="mk_t")
            ow_t = cpool.tile([128, 4, E], BF16, tag="ow", name="ow_t")
            nc.sync.dma_start(wk_t[127:128, 3, E - 8 : E],
                              xt_t[127:128, 3, LK - 8 : LK])
            nc.sync.dma_start(wk_t[:], wk[:].rearrange("(c p) e -> p c e", p=128))
            nc.scalar.dma_start(wv_t[127:128, 3, E - 8 : E],
                                wq_t[127:128, 3, E - 8 : E])
            nc.scalar.dma_start(wv_t[:], wv[:].rearrange("(c p) e -> p c e", p=128))
            nc.scalar.dma_start(mk_t[127:128, WSUM - 8 : WSUM],
                                wq_t[127:128, 3, E - 8 : E])
            nc.scalar.dma_start(mk_t[:], mk[:])
            nc.sync.dma_start(ow_t[127:128, 3, E - 8 : E],
                              xt_t[127:128, 3, LK - 8 : LK])
            nc.sync.dma_start(ow_t[:], ow[:].rearrange("(c p) e -> p c e", p=128))

            qb = bb_t[:, 0:4]
            kb = bb_t[:, 4:8]
            obT = bb_t[:, 8:12]
            vb3 = bb_t[:, 12:524].rearrange("p (h d) -> p h d", d=HD)

            # small constants
            dum_t = cpool.tile([128, E], BF16, tag="dum", name="dum_t")
            nc.vector.memset(dum_t[:], 0.0)
            ones_t = cpool.tile([1, 64], BF16, tag="ones", name="ones_t")
            nc.gpsimd.memset(ones_t[:], 1.0)

            # ACT table warmup: force the ln/exp table load during the DMA
            # front instead of at the first real exp.
            actw = spool.tile([1, 8], FP32, tag="actw", name="actw")
            nc.vector.memset(actw[:], 1.0)
            nc.scalar.activation(actw[0:1, 0:4], actw[0:1, 4:8], AF.Ln)
            nc.scalar.activation(actw[0:1, 4:8], actw[0:1, 0:4], AF.Exp)

            # HAM warmup: dummy matmuls with no DMA deps run while the
            # input DMAs stream in, so the PE clock gate is at 8/8 when the
            # first projection matmul issues.
            for i in range(5):
                psd = ps.tile([HD + 1, CH], FP32, name=f"dum{i}", tag="psO")
                nc.tensor.matmul(
                    psd[:, 0:CH], dum_t[:, 0 : HD + 1], dum_t[:, 0:CH],
                    start=True, stop=True,
                )

            def keepalive(p):
                # Accumulate 0 into pair p's open psO group: a real MATMUL
                # (not reorderable like LDWEIGHTS) that marks the PE busy in
                # HAM's activity window, so sparse stretches don't drop the
                # clock to 4/8. Only legal while psO(p) is mid-accumulation.
                nc.tensor.matmul(
                    pso_tiles[p][0][:, 0:64], dum_t[:, 0 : HD + 1],
                    dum_t[:, 0:64], start=False, stop=False,
                    skip_group_check=True,
                )

            qT = [None] * 4
            kT = [None] * 4
            v_t = [None] * NT
            vT = [None] * 4
            att_tiles = {}
            pso_tiles = {}

            def emit_qproj(p):
                psq = ps.tile([128, CH], FP32, tag="pc", name=f"psq{p}")
                for kk in range(4):
                    nc.tensor.matmul(
                        psq[:],
                        wq_t[:, kk, 128 * p : 128 * p + 128],
                        xt_t[:, kk, 128 : 128 + CH],
                        start=(kk == 0), stop=(kk == 3),
                    )
                q = ppool.tile([128, CH], BF16, tag=f"qT{p}", name=f"qT{p}")
                nc.vector.tensor_scalar_add(q[:], psq[:], qb[:, p : p + 1])
                qT[p] = q

            def emit_kproj(p):
                kT[p] = ppool.tile([128, LK], BF16, tag=f"kT{p}", name=f"kT{p}")
                for h, (a, b) in enumerate(((0, 512), (512, 768))):
                    psk = ps.tile([128, b - a], FP32, tag="pc", name=f"psk{p}_{h}")
                    for kk in range(4):
                        nc.tensor.matmul(
                            psk[:],
                            wk_t[:, kk, 128 * p : 128 * p + 128],
                            xt_t[:, kk, a:b],
                            start=(kk == 0), stop=(kk == 3),
                        )
                    nc.vector.tensor_scalar_add(
                        kT[p][:, a:b], psk[:], kb[:, p : p + 1]
                    )

            def emit_vproj(m):
                psv = ps.tile([128, E], FP32, tag="pc", name=f"psv{m}")
                for kk in range(4):
                    nc.tensor.matmul(
                        psv[:],
                        xt_t[:, kk, 128 * m : 128 * m + 128],
                        wv_t[:, kk, :],
                        start=(kk == 0), stop=(kk == 3),
                    )
                v = ppool.tile([128, H * (HD + 1)], BF16, tag=f"v{m}", name=f"v{m}")
                v3 = v[:].rearrange("p (h d) -> p h d", d=HD + 1)
                psv3 = psv[:].rearrange("p (h d) -> p h d", d=HD)
                nc.vector.tensor_tensor(v3[:, :, 0:HD], psv3, vb3, op=AluOpType.add)
                nc.gpsimd.memset(v3[:, :, HD : HD + 1], 1.0)
                v_t[m] = v

            def emit_qk_chunk(p, c):
                """QK chunk c (512 cols of the concat layout) for both halves:
                matmuls into a PSUM chunk, exp on ACT -> att bf16, mask mul."""
                if c == 0:
                    attA = apool.tile([128, WSUM], BF16, name=f"attA{p}", tag="attA")
                    attB = apool.tile([128, WSUM], BF16, name=f"attB{p}", tag="attB")
                    att_tiles[p] = (attA, attB)
                for hh in range(2):
                    att = att_tiles[p][hh]
                    pcx = ps.tile([128, 512], FP32, tag="pc", name=f"pc{p}_{c}_{hh}")
                    r = slice(64 * hh, 64 * hh + 64)
                    for (t, q0, w, d0) in CHUNK_MMS[c]:
                        nc.tensor.matmul(
                            pcx[:, d0 : d0 + w],
                            kT[p][r, 128 * t : 128 * t + 128],
                            qT[p][r, q0 : q0 + w],
                            start=True, stop=True,
                        )
                    nc.scalar.activation(
                        att[:, 512 * c : 512 * c + 512], pcx[:], AF.Exp
                    )
                    # mask: chunk 0 of mk holds 0xFFFF/0x0000 bit patterns
                    # and runs as a uint32 bitwise AND on DVE (bitwise is
                    # DVE-only and 32-bit only; halving the free size makes
                    # it ~1.8x cheaper than the fp multiply). Chunks 1-2
                    # hold fp 1.0/0.0 and multiply on GpSimd, which has no
                    # bitwise support but otherwise sits idle.
                    sl = att[:, 512 * c : 512 * c + 512]
                    if c == 0:
                        slu = sl.bitcast(U32)
                        nc.vector.tensor_tensor(
                            slu, slu,
                            mk_t[:, 512 * c : 512 * c + 512].bitcast(U32),
                            op=AluOpType.bitwise_and,
                        )
                    else:
                        nc.gpsimd.tensor_mul(
                            sl, sl, mk_t[:, 512 * c : 512 * c + 512]
                        )

            def emit_av(p, t):
                """AV matmuls for key tile t, both heads of pair p."""
                if t == 0:
                    pso_tiles[p] = (
                        ps.tile([HD + 1, CH], FP32, name=f"pso{2 * p}", tag="psO"),
                        ps.tile([HD + 1, CH], FP32, name=f"pso{2 * p + 1}", tag="psO"),
                    )
                w, qs, off = W_T[t], QS_T[t], OFF_T[t]
                for hh in range(2):
                    head = 2 * p + hh
                    nc.tensor.matmul(
                        pso_tiles[p][hh][:, qs : qs + w],
                        v_t[t][:, (HD + 1) * head : (HD + 1) * head + HD + 1],
                        att_tiles[p][hh][:, off : off + w],
                        start=(t == 0), stop=(t == NT - 1),
                        skip_group_check=True,
                    )

            def emit_norm(p):
                """Denominator -> reciprocal -> normalized values^T (bf16)."""
                psos = pso_tiles[p]
                # p<3: pc ring (interleaves with QK chunks). p==3: psO ring
                # (pair 2's slots are free by now) so all four psf partials
                # can hold the pc ring at the tail without deadlock.
                rbc = ps.tile([128, CH], FP32, tag="pc" if p < 3 else "psO",
                              name=f"rbc{p}")
                dens = []
                for hh in range(2):
                    den = spool.tile([1, CH], BF16, tag="den", name=f"den{2*p+hh}")
                    # extract the two den rows on different engines so they
                    # run in parallel (the serial den chain gates the tail)
                    if hh == 0:
                        nc.vector.tensor_copy(den[:], psos[hh][HD : HD + 1, :])
                    else:
                        nc.scalar.activation(den[:], psos[hh][HD : HD + 1, :],
                                             AF.Copy)
                    dens.append(den)
                for hh in range(2):
                    nc.tensor.matmul(
                        rbc[64 * hh : 64 * hh + 64, :], ones_t[:], dens[hh][:],
                        start=True, stop=True,
                    )
                lnv = spool.tile([128, CH], FP32, tag="lnv", name=f"lnv{p}")
                nc.scalar.activation(lnv[:], rbc[:], AF.Ln)
                rbs = spool.tile([128, CH], FP32, tag="rbs", name=f"rbs{p}")
                nc.scalar.activation(rbs[:], lnv[:], AF.Exp, scale=-1.0)
                vtn = ppool.tile([128, CH], BF16, tag=f"vT{p}", name=f"vT{p}")
                for hh in range(2):
                    nc.vector.tensor_mul(
                        vtn[64 * hh : 64 * hh + 64, :],
                        psos[hh][0:HD, :],
                        rbs[64 * hh : 64 * hh + 64, :],
                    )
                vT[p] = vtn

            # ================= emission schedule =================
            emit_qproj(0); emit_kproj(0)
            for c in range(3):
                emit_qk_chunk(0, c)
            emit_qproj(1); emit_kproj(1)
            emit_vproj(0); emit_vproj(1)
            for c in range(3):
                emit_qk_chunk(1, c)
            emit_av(0, 0); emit_av(0, 1)
            emit_qproj(2); emit_kproj(2)
            emit_vproj(2)
            emit_av(0, 2)
            emit_vproj(3)
            emit_av(0, 3)
            for c in range(3):
                emit_qk_chunk(2, c)
            emit_av(1, 0); emit_av(1, 1)
            emit_qproj(3); emit_kproj(3)
            emit_vproj(4)
            emit_av(0, 4)
            emit_av(1, 2)
            emit_vproj(5)
            emit_av(0, 5)          # psO(0) complete
            emit_norm(0)
            emit_av(1, 3)
            for c in range(3):
                emit_qk_chunk(3, c)
            emit_av(1, 4); emit_av(1, 5)   # psO(1) complete
            emit_norm(1)
            emit_av(2, 0); emit_av(2, 1)
            keepalive(2)
            emit_av(2, 2); emit_av(2, 3)
            keepalive(2)
            emit_av(2, 4); emit_av(2, 5)
            emit_norm(2)
            emit_av(3, 0); emit_av(3, 1)
            keepalive(3)
            emit_av(3, 2); emit_av(3, 3)
            keepalive(3)
            emit_av(3, 4); emit_av(3, 5)

            # ---- output projection (transposed: psfT[e, q]) ----
            # all four m-blocks accumulate p=0..2 while pair 3 normalizes;
            # only the p=3 matmul + bias-evict + store remain after norm(3).
            psf = [None] * 4

            def emit_psf_mms(m, ps_, pe_):
                if psf[m] is None:
                    psf[m] = ps.tile([128, CH], FP32, tag="pc", name=f"psf{m}")
                for p in range(ps_, pe_):
                    nc.tensor.matmul(
                        psf[m][:],
                        ow_t[:, p, 128 * m : 128 * m + 128],
                        vT[p][:],
                        start=(p == 0), stop=(p == 3),
                    )

            def emit_psf_out(m):
                fin = fpool.tile([128, E], BF16, tag="fin", name=f"fin{m}")
                if m % 2 == 0:
                    nc.vector.tensor_scalar_add(fin[:], psf[m][:], obT[:, m : m + 1])
                else:
                    nc.scalar.activation(fin[:], psf[m][:], AF.Identity,
                                         bias=obT[:, m : m + 1])
                dq = nc.sync if m % 2 == 0 else nc.scalar
                dq.dma_start(out[128 * m : 128 * m + 128, :], fin[:])

            for m in range(4):
                emit_psf_mms(m, 0, 3)
            emit_norm(3)
            for m in range(4):
                emit_psf_mms(m, 3, 4)
                emit_psf_out(m)

    return nc


_NC_CACHE = None


def _get_program():
    global _NC_CACHE
    if _NC_CACHE is None:
        _NC_CACHE = _build_program()
    return _NC_CACHE


def _make_in_maps(x, padding_mask, qkv_w, qkv_b, o_w, o_b):
    x = np.asarray(x, np.float32)
    pm = np.asarray(padding_mask)
    qkv_w = np.asarray(qkv_w, np.float32)
    qkv_b = np.asarray(qkv_b, np.float32)
    o_w = np.asarray(o_w, np.float32)
    o_b = np.asarray(o_b, np.float32)

    scale = np.float32(1.0 / np.sqrt(HD))
    # reference splits per-head: head h uses qkv rows [192h,192h+64) (q),
    # +64 (k), +128 (v)
    idx_q = np.concatenate([np.arange(3 * HD * h, 3 * HD * h + HD) for h in range(H)])
    idx_k = idx_q + HD
    idx_v = idx_q + 2 * HD

    wq = np.ascontiguousarray((qkv_w[idx_q] * scale).T.astype(BF))  # [IN, E]
    wk = np.ascontiguousarray(qkv_w[idx_k].T.astype(BF))
    wv = np.ascontiguousarray(qkv_w[idx_v].T.astype(BF))
    ow = np.ascontiguousarray(o_w.T.astype(BF))                     # [f, e]

    bigb = np.zeros((128, 524), np.float32)
    bigb[:, 0:4] = (qkv_b[idx_q] * scale).reshape(4, 128).T
    bigb[:, 4:8] = qkv_b[idx_k].reshape(4, 128).T
    bigb[:, 8:12] = o_b.reshape(4, 128).T
    bigb[:, 12:524] = np.broadcast_to(qkv_b[idx_v][None, :], (128, E))

    xb = x.astype(BF)

    j = np.arange(128)[:, None]
    in_maps = []
    for c in range(8):
        b, ch = divmod(c, 4)
        s0 = CH * ch
        lo, hi = max(0, s0 - HW), min(S, s0 + CH + HW)
        xpad = np.zeros((LK, IN_DIM), BF)
        xpad[lo - (s0 - HW) : hi - (s0 - HW)] = xb[b, lo:hi]
        xt = np.ascontiguousarray(xpad.T)                     # [IN, LK] bf16

        mask = np.zeros((128, WSUM), np.float32)
        for t in range(NT):
            w, qs, off = W_T[t], QS_T[t], OFF_T[t]
            lk = 128 * t + j                                  # [128,1] local key
            q = qs + np.arange(w)[None, :]                    # [1,w] local query
            band = (q <= lk) & (lk <= q + 2 * HW)
            gk = s0 - HW + lk                                 # global key index
            valid = (gk >= 0) & (gk < S)
            pmk = pm[b, np.clip(gk, 0, S - 1)] != 0
            mask[:, off : off + w] = (band & valid & pmk).astype(np.float32)

        # chunk 0 (cols 0:512) as 0xFFFF/0 bit patterns for the DVE AND;
        # chunks 1-2 as fp 1.0/0.0 bf16 for the GpSimd multiply
        mkb = mask.astype(BF)
        bits = np.where(mask[:, 0:512] != 0, np.uint16(0xFFFF), np.uint16(0))
        mkb[:, 0:512] = bits.view(BF)
        in_maps.append(
            {"xt": xt, "wq": wq, "wk": wk, "wv": wv, "ow": ow,
             "mk": mkb, "bigb": bigb}
        )
    return in_maps


def _run(x, padding_mask, qkv_w, qkv_b, o_w, o_b, trace=False, tmpdir=None):
    nc = _get_program()
    in_maps = _make_in_maps(x, padding_mask, qkv_w, qkv_b, o_w, o_b)
    res = run_bass_kernel_spmd(
        nc, in_maps, core_ids=list(range(8)), trace=trace, tmpdir=tmpdir
    )
    o = np.empty((B, S, E), np.float32)
    for c in range(8):
        b, ch = divmod(c, 4)
        o[b, CH * ch : CH * ch + CH, :] = (
            res.results[c]["out"].astype(np.float32).T
        )
    # fully-masked query rows: att = 0 -> output is exactly the bias
    pm = np.asarray(padding_mask)
    if (pm == 0).any():
        o[pm == 0] = np.asarray(o_b, np.float32)
    return o, res


def kernel(x, padding_mask, qkv_w, qkv_b, o_w, o_b, window_size, num_heads):
    assert int(window_size) == WS and int(num_heads) == H
    assert tuple(np.asarray(x).shape) == (B, S, IN_DIM)
    o, _ = _run(x, padding_mask, qkv_w, qkv_b, o_w, o_b)
    return o


# revision 25
# speedup vs baseline: 1.1636x; 1.1636x over previous
"""Sliding-window multi-head attention for Trainium2, 8-core SPMD.

Sharding: sequence-parallel. B=2 batches x 4 chunks of 512 queries = 8 cores.
Each core computes QKV projections for its chunk (+128-row halo for K/V),
banded attention (window 256 -> band |j-s|<=128), and the output projection
for its 512 rows. No collectives; host concatenates the 8 output chunks.

v2: full bf16 pipeline (weights/activations bf16, fp32 PSUM accum).
 - bf16 matmuls: 1 cyc/row at any width (fp32r paid 4x below N=256), FWL
   halves LDWEIGHTS, and DMA bytes halve.
 - QK scores^T are written in three 512-col PSUM chunks per (pair, half) so
   exp runs as 3 big ACT ops instead of 6 small ones.
 - Engine rebalance around "GpSimd cannot read PSUM": all PSUM evictions on
   DVE/ACT, mask-muls mostly on GpSimd (SBUF-only), some on DVE (bf16 2x).
 - Denominators: free via ones-column in AV; recip = exp(-ln(den)) on ACT;
   broadcast to 64 partitions via one K=2 matmul per pair (sel matrix).
 - O-projection emitted transposed (out^T[e,q]) so the bias add is a
   per-partition ACT Identity+bias; host transposes back.
 - Projection work is interleaved with attention so ACT starts exps ~6us in.
"""

import numpy as np
import ml_dtypes

import concourse.bass as bass
import concourse.tile as tile
from concourse import mybir
from concourse.alu_op_type import AluOpType
from concourse.vector_clock import ScopedClock
from concourse.bass_utils import run_bass_kernel_spmd

FP32 = mybir.dt.float32
BF16 = mybir.dt.bfloat16
U32 = mybir.dt.uint32
AF = mybir.ActivationFunctionType
BF = ml_dtypes.bfloat16

# Problem constants (hardcoded per contract)
B, S, IN_DIM, E = 2, 2048, 512, 512
H, HD = 8, 64
WS, HW = 256, 128
CH = 512          # own queries per core
LK = 768          # local keys per core (chunk + 128 halo each side)
NT = 6            # key tiles of 128
W_T = [128, 256, 384, 384, 256, 128]   # valid query-span width per key tile
QS_T = [0, 0, 0, 128, 256, 384]        # local query start per key tile
OFF_T = [0, 128, 384, 768, 1152, 1408]  # column offset in the concat layout
WSUM = 1536

# QK matmuls per (pair, half), grouped by 512-col PSUM chunk:
# (tile, query_start, width, dst_col_in_chunk)
CHUNK_MMS = [
    [(0, 0, 128, 0), (1, 0, 256, 128), (2, 0, 128, 384)],
    [(2, 128, 256, 0), (3, 128, 256, 256)],
    [(3, 384, 128, 0), (4, 256, 256, 128), (5, 384, 128, 384)],
]

_MAX_WAITS = 1
_patched = False


def _split_sync_waits(nc):
    """This container's walrus accepts only 1 sync-wait per instruction.
    Move extra waits onto nofuse NOPs inserted just before, on the same
    engine sequencer (in-order execution makes this equivalent)."""
    n_split = 0
    for fn in nc.m.functions:
        for bb in fn.blocks:
            insts = list(bb.instructions)
            out = []
            for inst in insts:
                si = inst.sync_info
                if si is not None and len(si.on_wait) > _MAX_WAITS:
                    waits = list(si.on_wait)
                    extra, keep = waits[:-_MAX_WAITS], waits[-_MAX_WAITS:]
                    for j in range(0, len(extra), _MAX_WAITS):
                        out.append(
                            mybir.InstNoOp(
                                name=f"{inst.name}-sw{j}",
                                engine=inst.engine,
                                bass_nofuse=True,
                                sync_info=mybir.SyncInfo(
                                    on_wait=extra[j : j + _MAX_WAITS], on_update=[]
                                ),
                            )
                        )
                    inst.sync_info = mybir.SyncInfo(
                        on_wait=keep, on_update=list(si.on_update)
                    )
                    n_split += 1
                out.append(inst)
            if len(out) != len(insts):
                try:
                    bb.instructions = out
                except Exception:
                    bb.instructions[:] = out
    return n_split


def _patch_tile_drain():
    global _patched
    if _patched:
        return
    _patched = True

    def _drain_and_barrier(self, tick_clock, wait_clock):
        nc = self.nc
        drain_inst = nc.sync.drain()
        wait_clock.add_sem_waits(
            drain_inst.ins, ScopedClock({None: tick_clock.global_clock})
        )
        nc.all_engine_barrier()
        assert self.sems is not None
        popped = nc._tile_sem_poison_stack.pop()
        assert popped is self._sem_poison
        nc.clear_and_free_semaphores(list(self.sems.allocated().values()))
        nc.all_engine_barrier()
        _split_sync_waits(nc)

    tile.TileContext._drain_and_barrier = _drain_and_barrier


def _build_program():
    _patch_tile_drain()
    nc = bass.Bass("TRN2", target_bir_lowering=False, debug=False)

    xt = nc.dram_tensor("xt", [IN_DIM, LK], BF16, kind="ExternalInput")
    wq = nc.dram_tensor("wq", [IN_DIM, E], BF16, kind="ExternalInput")
    wk = nc.dram_tensor("wk", [IN_DIM, E], BF16, kind="ExternalInput")
    wv = nc.dram_tensor("wv", [IN_DIM, E], BF16, kind="ExternalInput")
    ow = nc.dram_tensor("ow", [E, E], BF16, kind="ExternalInput")
    mk = nc.dram_tensor("mk", [128, WSUM], BF16, kind="ExternalInput")
    # bigb fp32 [128, 524]: cols 0:4 qb per pair, 4:8 kb, 8:12 obT, 12:524 vb
    bigb = nc.dram_tensor("bigb", [128, 524], FP32, kind="ExternalInput")
    out = nc.dram_tensor("out", [E, CH], BF16, kind="ExternalOutput")

    with tile.TileContext(nc) as tc:
        with (
            tc.tile_pool(name="const", bufs=1) as cpool,
            tc.tile_pool(name="proj", bufs=1) as ppool,
            tc.tile_pool(name="att", bufs=4) as apool,
            tc.tile_pool(name="small", bufs=2) as spool,
            tc.tile_pool(name="fin", bufs=2) as fpool,
            tc.tile_pool(name="ps", bufs=4, space="PSUM") as ps,
        ):
            # ---- input DMAs ----
            # The SDMA engines round-robin between all in-flight DMAs at
            # packet granularity, so everything dispatched together finishes
            # together. To get xt+wq (the q-projection inputs) early, gate
            # the remaining transfers behind tiny SBUF->SBUF reads of the
            # freshly landed tiles (the gate blocks its queue's dispatch
            # until the previous transfer completes).
            # First wave (ungated, transfers share bandwidth and finish
            # together ~5us in): everything the projection front needs.
            bb_t = cpool.tile([128, 524], FP32, tag="bigb", name="bb_t")
            nc.sync.dma_start(bb_t[:], bigb[:])
            xt_t = cpool.tile([128, 4, LK], BF16, tag="xt", name="xt_t")
            nc.sync.dma_start(xt_t[:], xt[:].rearrange("(c p) j -> p c j", p=128))
            wq_t = cpool.tile([128, 4, E], BF16, tag="wq", name="wq_t")
            nc.scalar.dma_start(wq_t[:], wq[:].rearrange("(c p) e -> p c e", p=128))
            wk_t = cpool.tile([128, 4, E], BF16, tag="wk", name="wk_t")
            nc.sync.dma_start(wk_t[:], wk[:].rearrange("(c p) e -> p c e", p=128))
            # Second wave (mk/wv/ow), gated behind the first: a tiny DMA
            # writes each tile's corner sourced from a first-wave tile, and
            # the WAW overlap delays the real transfer. (Plain queue order
            # can't prioritize: every dma_start lands on its own HW ring and
            # all rings round-robin at packet granularity.)
            wv_t = cpool.tile([128, 4, E], BF16, tag="wv", name="wv_t")
            mk_t = cpool.tile([128, WSUM], BF16, tag="mk", name="mk_t")
            ow_t = cpool.tile([128, 4, E], BF16, tag="ow", name="ow_t")
            nc.scalar.dma_start(mk_t[127:128, WSUM - 8 : WSUM],
                                wq_t[127:128, 3, E - 8 : E])
            nc.scalar.dma_start(mk_t[:], mk[:])
            nc.scalar.dma_start(wv_t[127:128, 3, E - 8 : E],
                                wq_t[127:128, 3, E - 8 : E])
            nc.scalar.dma_start(wv_t[:], wv[:].rearrange("(c p) e -> p c e", p=128))
            nc.sync.dma_start(ow_t[127:128, 3, E - 8 : E],
                              wk_t[127:128, 3, E - 8 : E])
            nc.sync.dma_start(ow_t[:], ow[:].rearrange("(c p) e -> p c e", p=128))

            qb = bb_t[:, 0:4]
            kb = bb_t[:, 4:8]
            obT = bb_t[:, 8:12]
            vb3 = bb_t[:, 12:524].rearrange("p (h d) -> p h d", d=HD)

            # small constants
            dum_t = cpool.tile([128, E], BF16, tag="dum", name="dum_t")
            nc.vector.memset(dum_t[:], 0.0)
            ones_t = cpool.tile([1, 64], BF16, tag="ones", name="ones_t")
            nc.gpsimd.memset(ones_t[:], 1.0)

            # ACT table warmup: force the ln/exp table load during the DMA
            # front instead of at the first real exp.
            actw = spool.tile([1, 8], FP32, tag="actw", name="actw")
            nc.vector.memset(actw[:], 1.0)
            nc.scalar.activation(actw[0:1, 0:4], actw[0:1, 4:8], AF.Ln)
            nc.scalar.activation(actw[0:1, 4:8], actw[0:1, 0:4], AF.Exp)

            # HAM warmup: dummy matmuls with no DMA deps run while the
            # input DMAs stream in, so the PE clock gate is at 8/8 when the
            # first projection matmul issues.
            for i in range(5):
                psd = ps.tile([HD + 1, CH], FP32, name=f"dum{i}", tag="psO")
                nc.tensor.matmul(
                    psd[:, 0:CH], dum_t[:, 0 : HD + 1], dum_t[:, 0:CH],
                    start=True, stop=True,
                )

            def keepalive(p):
                # Accumulate 0 into pair p's open psO group: a real MATMUL
                # (not reorderable like LDWEIGHTS) that marks the PE busy in
                # HAM's activity window, so sparse stretches don't drop the
                # clock to 4/8. Only legal while psO(p) is mid-accumulation.
                nc.tensor.matmul(
                    pso_tiles[p][0][:, 0:64], dum_t[:, 0 : HD + 1],
                    dum_t[:, 0:64], start=False, stop=False,
                    skip_group_check=True,
                )

            qT = [None] * 4
            kT = [None] * 4
            v_t = [None] * NT
            vT = [None] * 4
            att_tiles = {}
            pso_tiles = {}

            def emit_qproj(p):
                psq = ps.tile([128, CH], FP32, tag="pc", name=f"psq{p}")
                for kk in range(4):
                    nc.tensor.matmul(
                        psq[:],
                        wq_t[:, kk, 128 * p : 128 * p + 128],
                        xt_t[:, kk, 128 : 128 + CH],
                        start=(kk == 0), stop=(kk == 3),
                    )
                q = ppool.tile([128, CH], BF16, tag=f"qT{p}", name=f"qT{p}")
                nc.vector.tensor_scalar_add(q[:], psq[:], qb[:, p : p + 1])
                qT[p] = q

            def emit_kproj(p):
                kT[p] = ppool.tile([128, LK], BF16, tag=f"kT{p}", name=f"kT{p}")
                for h, (a, b) in enumerate(((0, 512), (512, 768))):
                    psk = ps.tile([128, b - a], FP32, tag="pc", name=f"psk{p}_{h}")
                    for kk in range(4):
                        nc.tensor.matmul(
                            psk[:],
                            wk_t[:, kk, 128 * p : 128 * p + 128],
                            xt_t[:, kk, a:b],
                            start=(kk == 0), stop=(kk == 3),
                        )
                    nc.vector.tensor_scalar_add(
                        kT[p][:, a:b], psk[:], kb[:, p : p + 1]
                    )

            def emit_vproj(m):
                psv = ps.tile([128, E], FP32, tag="pc", name=f"psv{m}")
                for kk in range(4):
                    nc.tensor.matmul(
                        psv[:],
                        xt_t[:, kk, 128 * m : 128 * m + 128],
                        wv_t[:, kk, :],
                        start=(kk == 0), stop=(kk == 3),
                    )
                v = ppool.tile([128, H * (HD + 1)], BF16, tag=f"v{m}", name=f"v{m}")
                v3 = v[:].rearrange("p (h d) -> p h d", d=HD + 1)
                psv3 = psv[:].rearrange("p (h d) -> p h d", d=HD)
                nc.vector.tensor_tensor(v3[:, :, 0:HD], psv3, vb3, op=AluOpType.add)
                nc.gpsimd.memset(v3[:, :, HD : HD + 1], 1.0)
                v_t[m] = v

            def emit_qk_chunk(p, c):
                """QK chunk c (512 cols of the concat layout) for both halves:
                matmuls into a PSUM chunk, exp on ACT -> att bf16, mask mul."""
                if c == 0:
                    attA = apool.tile([128, WSUM], BF16, name=f"attA{p}", tag="attA")
                    attB = apool.tile([128, WSUM], BF16, name=f"attB{p}", tag="attB")
                    att_tiles[p] = (attA, attB)
                for hh in range(2):
                    att = att_tiles[p][hh]
                    pcx = ps.tile([128, 512], FP32, tag="pc", name=f"pc{p}_{c}_{hh}")
                    r = slice(64 * hh, 64 * hh + 64)
                    for (t, q0, w, d0) in CHUNK_MMS[c]:
                        nc.tensor.matmul(
                            pcx[:, d0 : d0 + w],
                            kT[p][r, 128 * t : 128 * t + 128],
                            qT[p][r, q0 : q0 + w],
                            start=True, stop=True,
                        )
                    nc.scalar.activation(
                        att[:, 512 * c : 512 * c + 512], pcx[:], AF.Exp
                    )
                    # mask: chunk 0 of mk holds 0xFFFF/0x0000 bit patterns
                    # and runs as a uint32 bitwise AND on DVE (bitwise is
                    # DVE-only and 32-bit only; halving the free size makes
                    # it ~1.8x cheaper than the fp multiply). Chunks 1-2
                    # hold fp 1.0/0.0 and multiply on GpSimd, which has no
                    # bitwise support but otherwise sits idle.
                    sl = att[:, 512 * c : 512 * c + 512]
                    if c == 0:
                        slu = sl.bitcast(U32)
                        nc.vector.tensor_tensor(
                            slu, slu,
                            mk_t[:, 512 * c : 512 * c + 512].bitcast(U32),
                            op=AluOpType.bitwise_and,
                        )
                    else:
                        nc.gpsimd.tensor_mul(
                            sl, sl, mk_t[:, 512 * c : 512 * c + 512]
                        )

            def emit_av(p, t):
                """AV matmuls for key tile t, both heads of pair p."""
                if t == 0:
                    pso_tiles[p] = (
                        ps.tile([HD + 1, CH], FP32, name=f"pso{2 * p}", tag="psO"),
                        ps.tile([HD + 1, CH], FP32, name=f"pso{2 * p + 1}", tag="psO"),
                    )
                w, qs, off = W_T[t], QS_T[t], OFF_T[t]
                for hh in range(2):
                    head = 2 * p + hh
                    nc.tensor.matmul(
                        pso_tiles[p][hh][:, qs : qs + w],
                        v_t[t][:, (HD + 1) * head : (HD + 1) * head + HD + 1],
                        att_tiles[p][hh][:, off : off + w],
                        start=(t == 0), stop=(t == NT - 1),
                        skip_group_check=True,
                    )

            def emit_norm(p):
                """Denominator -> reciprocal -> normalized values^T (bf16)."""
                psos = pso_tiles[p]
                # p<3: pc ring (interleaves with QK chunks). p==3: psO ring
                # (pair 2's slots are free by now) so all four psf partials
                # can hold the pc ring at the tail without deadlock.
                rbc = ps.tile([128, CH], FP32, tag="pc" if p < 3 else "psO",
                              name=f"rbc{p}")
                dens = []
                for hh in range(2):
                    den = spool.tile([1, CH], BF16, tag="den", name=f"den{2*p+hh}")
                    # extract the two den rows on different engines so they
                    # run in parallel (the serial den chain gates the tail)
                    if hh == 0:
                        nc.vector.tensor_copy(den[:], psos[hh][HD : HD + 1, :])
                    else:
                        nc.scalar.activation(den[:], psos[hh][HD : HD + 1, :],
                                             AF.Copy)
                    dens.append(den)
                for hh in range(2):
                    nc.tensor.matmul(
                        rbc[64 * hh : 64 * hh + 64, :], ones_t[:], dens[hh][:],
                        start=True, stop=True,
                    )
                lnv = spool.tile([128, CH], FP32, tag="lnv", name=f"lnv{p}")
                nc.scalar.activation(lnv[:], rbc[:], AF.Ln)
                rbs = spool.tile([128, CH], FP32, tag="rbs", name=f"rbs{p}")
                nc.scalar.activation(rbs[:], lnv[:], AF.Exp, scale=-1.0)
                vtn = ppool.tile([128, CH], BF16, tag=f"vT{p}", name=f"vT{p}")
                for hh in range(2):
                    nc.vector.tensor_mul(
                        vtn[64 * hh : 64 * hh + 64, :],
                        psos[hh][0:HD, :],
                        rbs[64 * hh : 64 * hh + 64, :],
                    )
                vT[p] = vtn

            # ================= emission schedule =================
            emit_qproj(0); emit_kproj(0)
            for c in range(3):
                emit_qk_chunk(0, c)
            emit_qproj(1); emit_kproj(1)
            emit_vproj(0); emit_vproj(1)
            for c in range(3):
                emit_qk_chunk(1, c)
            emit_av(0, 0); emit_av(0, 1)
            emit_qproj(2); emit_kproj(2)
            emit_vproj(2)
            emit_av(0, 2)
            emit_vproj(3)
            emit_av(0, 3)
            for c in range(3):
                emit_qk_chunk(2, c)
            emit_av(1, 0); emit_av(1, 1)
            emit_qproj(3); emit_kproj(3)
            emit_vproj(4)
            emit_av(0, 4)
            emit_av(1, 2)
            emit_vproj(5)
            emit_av(0, 5)          # psO(0) complete
            emit_norm(0)
            emit_av(1, 3)
            for c in range(3):
                emit_qk_chunk(3, c)
            emit_av(1, 4); emit_av(1, 5)   # psO(1) complete
            emit_norm(1)
            emit_av(2, 0); emit_av(2, 1)
            keepalive(2)
            emit_av(2, 2); emit_av(2, 3)
            keepalive(2)
            emit_av(2, 4); emit_av(2, 5)
            emit_norm(2)
            emit_av(3, 0); emit_av(3, 1)
            keepalive(3)
            emit_av(3, 2); emit_av(3, 3)
            keepalive(3)
            emit_av(3, 4); emit_av(3, 5)

            # ---- output projection (transposed: psfT[e, q]) ----
            # all four m-blocks accumulate p=0..2 while pair 3 normalizes;
            # only the p=3 matmul + bias-evict + store remain after norm(3).
            psf = [None] * 4

            def emit_psf_mms(m, ps_, pe_):
                if psf[m] is None:
                    psf[m] = ps.tile([128, CH], FP32, tag="pc", name=f"psf{m}")
                for p in range(ps_, pe_):
                    nc.tensor.matmul(
                        psf[m][:],
                        ow_t[:, p, 128 * m : 128 * m + 128],
                        vT[p][:],
                        start=(p == 0), stop=(p == 3),
                    )

            def emit_psf_out(m):
                fin = fpool.tile([128, E], BF16, tag="fin", name=f"fin{m}")
                if m % 2 == 0:
                    nc.vector.tensor_scalar_add(fin[:], psf[m][:], obT[:, m : m + 1])
                else:
                    nc.scalar.activation(fin[:], psf[m][:], AF.Identity,
                                         bias=obT[:, m : m + 1])
                dq = nc.sync if m % 2 == 0 else nc.scalar
                dq.dma_start(out[128 * m : 128 * m + 128, :], fin[:])

            for m in range(4):
                emit_psf_mms(m, 0, 3)
            emit_norm(3)
            for m in range(4):
                emit_psf_mms(m, 3, 4)
                emit_psf_out(m)

    return nc


_NC_CACHE = None


def _get_program():
    global _NC_CACHE
    if _NC_CACHE is None:
        _NC_CACHE = _build_program()
    return _NC_CACHE


def _make_in_maps(x, padding_mask, qkv_w, qkv_b, o_w, o_b):
    x = np.asarray(x, np.float32)
    pm = np.asarray(padding_mask)
    qkv_w = np.asarray(qkv_w, np.float32)
    qkv_b = np.asarray(qkv_b, np.float32)
    o_w = np.asarray(o_w, np.float32)
    o_b = np.asarray(o_b, np.float32)

    scale = np.float32(1.0 / np.sqrt(HD))
    # reference splits per-head: head h uses qkv rows [192h,192h+64) (q),
    # +64 (k), +128 (v)
    idx_q = np.concatenate([np.arange(3 * HD * h, 3 * HD * h + HD) for h in range(H)])
    idx_k = idx_q + HD
    idx_v = idx_q + 2 * HD

    wq = np.ascontiguousarray((qkv_w[idx_q] * scale).T.astype(BF))  # [IN, E]
    wk = np.ascontiguousarray(qkv_w[idx_k].T.astype(BF))
    wv = np.ascontiguousarray(qkv_w[idx_v].T.astype(BF))
    ow = np.ascontiguousarray(o_w.T.astype(BF))                     # [f, e]

    bigb = np.zeros((128, 524), np.float32)
    bigb[:, 0:4] = (qkv_b[idx_q] * scale).reshape(4, 128).T
    bigb[:, 4:8] = qkv_b[idx_k].reshape(4, 128).T
    bigb[:, 8:12] = o_b.reshape(4, 128).T
    bigb[:, 12:524] = np.broadcast_to(qkv_b[idx_v][None, :], (128, E))

    xb = x.astype(BF)

    j = np.arange(128)[:, None]
    in_maps = []
    for c in range(8):
        b, ch = divmod(c, 4)
        s0 = CH * ch
        lo, hi = max(0, s0 - HW), min(S, s0 + CH + HW)
        xpad = np.zeros((LK, IN_DIM), BF)
        xpad[lo - (s0 - HW) : hi - (s0 - HW)] = xb[b, lo:hi]
        xt = np.ascontiguousarray(xpad.T)                     # [IN, LK] bf16

        mask = np.zeros((128, WSUM), np.float32)
        for t in range(NT):
            w, qs, off = W_T[t], QS_T[t], OFF_T[t]
            lk = 128 * t + j                                  # [128,1] local key
            q = qs + np.arange(w)[None, :]                    # [1,w] local query
            band = (q <= lk) & (lk <= q + 2 * HW)
            gk = s0 - HW + lk                                 # global key index
            valid = (gk >= 0) & (gk < S)
            pmk = pm[b, np.clip(gk, 0, S - 1)] != 0
            mask[:, off : off + w] = (band & valid & pmk).astype(np.float32)

        # chunk 0 (cols 0:512) as 0xFFFF/0 bit patterns for the DVE AND;
        # chunks 1-2 as fp 1.0/0.0 bf16 for the GpSimd multiply
        mkb = mask.astype(BF)
        bits = np.where(mask[:, 0:512] != 0, np.uint16(0xFFFF), np.uint16(0))
        mkb[:, 0:512] = bits.view(BF)
        in_maps.append(
            {"xt": xt, "wq": wq, "wk": wk, "wv": wv, "ow": ow,
             "mk": mkb, "bigb": bigb}
        )
    return in_maps


def _run(x, padding_mask, qkv_w, qkv_b, o_w, o_b, trace=False, tmpdir=None):
    nc = _get_program()
    in_maps = _make_in_maps(x, padding_mask, qkv_w, qkv_b, o_w, o_b)
    res = run_bass_kernel_spmd(
        nc, in_maps, core_ids=list(range(8)), trace=trace, tmpdir=tmpdir
    )
    o = np.empty((B, S, E), np.float32)
    for c in range(8):
        b, ch = divmod(c, 4)
        o[b, CH * ch : CH * ch + CH, :] = (
            res.results[c]["out"].astype(np.float32).T
        )
    # fully-masked query rows: att = 0 -> output is exactly the bias
    pm = np.asarray(padding_mask)
    if (pm == 0).any():
        o[pm == 0] = np.asarray(o_b, np.float32)
    return o, res


def kernel(x, padding_mask, qkv_w, qkv_b, o_w, o_b, window_size, num_heads):
    assert int(window_size) == WS and int(num_heads) == H
    assert tuple(np.asarray(x).shape) == (B, S, IN_DIM)
    o, _ = _run(x, padding_mask, qkv_w, qkv_b, o_w, o_b)
    return o


# revision 32
# speedup vs baseline: 1.2209x; 1.0493x over previous
"""Sliding-window multi-head attention for Trainium2, 8-core SPMD.

Sharding: sequence-parallel. B=2 batches x 4 chunks of 512 queries = 8 cores.
Each core computes QKV projections for its chunk (+128-row halo for K/V),
banded attention (window 256 -> band |j-s|<=128), and the output projection
for its 512 rows. No collectives; host concatenates the 8 output chunks.

v2: full bf16 pipeline (weights/activations bf16, fp32 PSUM accum).
 - bf16 matmuls: 1 cyc/row at any width (fp32r paid 4x below N=256), FWL
   halves LDWEIGHTS, and DMA bytes halve.
 - QK scores^T are written in three 512-col PSUM chunks per (pair, half) so
   exp runs as 3 big ACT ops instead of 6 small ones.
 - Engine rebalance around "GpSimd cannot read PSUM": all PSUM evictions on
   DVE/ACT, mask-muls mostly on GpSimd (SBUF-only), some on DVE (bf16 2x).
 - Denominators: free via ones-column in AV; recip = exp(-ln(den)) on ACT;
   broadcast to 64 partitions via one K=2 matmul per pair (sel matrix).
 - O-projection emitted transposed (out^T[e,q]) so the bias add is a
   per-partition ACT Identity+bias; host transposes back.
 - Projection work is interleaved with attention so ACT starts exps ~6us in.
"""

import numpy as np
import ml_dtypes

import concourse.bass as bass
import concourse.tile as tile
from concourse import mybir
from concourse.alu_op_type import AluOpType
from concourse.vector_clock import ScopedClock
from concourse.bass_utils import run_bass_kernel_spmd

FP32 = mybir.dt.float32
BF16 = mybir.dt.bfloat16
U32 = mybir.dt.uint32
AF = mybir.ActivationFunctionType
BF = ml_dtypes.bfloat16

# Problem constants (hardcoded per contract)
B, S, IN_DIM, E = 2, 2048, 512, 512
H, HD = 8, 64
WS, HW = 256, 128
CH = 512          # own queries per core
LK = 768          # local keys per core (chunk + 128 halo each side)
NT = 6            # key tiles of 128
W_T = [128, 256, 384, 384, 256, 128]   # valid query-span width per key tile
QS_T = [0, 0, 0, 128, 256, 384]        # local query start per key tile
OFF_T = [0, 128, 384, 768, 1152, 1408]  # column offset in the concat layout
WSUM = 1536
MKW = 2560      # mask cols: 0:512 bits, 512:1536 fp, 1536:2560 bits (pair-3)

# QK matmuls per (pair, half), grouped by 512-col PSUM chunk:
# (tile, query_start, width, dst_col_in_chunk)
CHUNK_MMS = [
    [(0, 0, 128, 0), (1, 0, 256, 128), (2, 0, 128, 384)],
    [(2, 128, 256, 0), (3, 128, 256, 256)],
    [(3, 384, 128, 0), (4, 256, 256, 128), (5, 384, 128, 384)],
]

_MAX_WAITS = 1
_patched = False


def _split_sync_waits(nc):
    """This container's walrus accepts only 1 sync-wait per instruction.
    Move extra waits onto nofuse NOPs inserted just before, on the same
    engine sequencer (in-order execution makes this equivalent)."""
    n_split = 0
    for fn in nc.m.functions:
        for bb in fn.blocks:
            insts = list(bb.instructions)
            out = []
            for inst in insts:
                si = inst.sync_info
                if si is not None and len(si.on_wait) > _MAX_WAITS:
                    waits = list(si.on_wait)
                    extra, keep = waits[:-_MAX_WAITS], waits[-_MAX_WAITS:]
                    for j in range(0, len(extra), _MAX_WAITS):
                        out.append(
                            mybir.InstNoOp(
                                name=f"{inst.name}-sw{j}",
                                engine=inst.engine,
                                bass_nofuse=True,
                                sync_info=mybir.SyncInfo(
                                    on_wait=extra[j : j + _MAX_WAITS], on_update=[]
                                ),
                            )
                        )
                    inst.sync_info = mybir.SyncInfo(
                        on_wait=keep, on_update=list(si.on_update)
                    )
                    n_split += 1
                out.append(inst)
            if len(out) != len(insts):
                try:
                    bb.instructions = out
                except Exception:
                    bb.instructions[:] = out
    return n_split


def _patch_tile_drain():
    global _patched
    if _patched:
        return
    _patched = True

    def _drain_and_barrier(self, tick_clock, wait_clock):
        nc = self.nc
        drain_inst = nc.sync.drain()
        wait_clock.add_sem_waits(
            drain_inst.ins, ScopedClock({None: tick_clock.global_clock})
        )
        nc.all_engine_barrier()
        assert self.sems is not None
        popped = nc._tile_sem_poison_stack.pop()
        assert popped is self._sem_poison
        nc.clear_and_free_semaphores(list(self.sems.allocated().values()))
        nc.all_engine_barrier()
        _split_sync_waits(nc)

    tile.TileContext._drain_and_barrier = _drain_and_barrier


def _build_program():
    _patch_tile_drain()
    nc = bass.Bass("TRN2", target_bir_lowering=False, debug=False)

    xt = nc.dram_tensor("xt", [IN_DIM, LK], BF16, kind="ExternalInput")
    wq = nc.dram_tensor("wq", [IN_DIM, E], BF16, kind="ExternalInput")
    wk = nc.dram_tensor("wk", [IN_DIM, E], BF16, kind="ExternalInput")
    wv = nc.dram_tensor("wv", [IN_DIM, E], BF16, kind="ExternalInput")
    ow = nc.dram_tensor("ow", [E, E], BF16, kind="ExternalInput")
    mk = nc.dram_tensor("mk", [128, MKW], BF16, kind="ExternalInput")
    # bigb fp32 [128, 524]: cols 0:4 qb per pair, 4:8 kb, 8:12 obT, 12:524 vb
    bigb = nc.dram_tensor("bigb", [128, 524], FP32, kind="ExternalInput")
    out = nc.dram_tensor("out", [E, CH], BF16, kind="ExternalOutput")

    with tile.TileContext(nc) as tc:
        with (
            tc.tile_pool(name="const", bufs=1) as cpool,
            tc.tile_pool(name="proj", bufs=1) as ppool,
            tc.tile_pool(name="att", bufs=4) as apool,
            tc.tile_pool(name="small", bufs=2) as spool,
            tc.tile_pool(name="fin", bufs=2) as fpool,
            tc.tile_pool(name="ps", bufs=4, space="PSUM") as ps,
        ):
            # ---- input DMAs ----
            # The SDMA engines round-robin between all in-flight DMAs at
            # packet granularity, so everything dispatched together finishes
            # together. To get xt+wq (the q-projection inputs) early, gate
            # the remaining transfers behind tiny SBUF->SBUF reads of the
            # freshly landed tiles (the gate blocks its queue's dispatch
            # until the previous transfer completes).
            # First wave (ungated, transfers share bandwidth and finish
            # together ~5us in): everything the projection front needs.
            bb_t = cpool.tile([128, 524], FP32, tag="bigb", name="bb_t")
            nc.sync.dma_start(bb_t[:], bigb[:])
            xt_t = cpool.tile([128, 4, LK], BF16, tag="xt", name="xt_t")
            nc.sync.dma_start(xt_t[:], xt[:].rearrange("(c p) j -> p c j", p=128))
            wq_t = cpool.tile([128, 4, E], BF16, tag="wq", name="wq_t")
            nc.scalar.dma_start(wq_t[:], wq[:].rearrange("(c p) e -> p c e", p=128))
            wk_t = cpool.tile([128, 4, E], BF16, tag="wk", name="wk_t")
            nc.sync.dma_start(wk_t[:], wk[:].rearrange("(c p) e -> p c e", p=128))
            # Second wave (mk/wv/ow), gated behind the first: a tiny DMA
            # writes each tile's corner sourced from a first-wave tile, and
            # the WAW overlap delays the real transfer. (Plain queue order
            # can't prioritize: every dma_start lands on its own HW ring and
            # all rings round-robin at packet granularity.)
            wv_t = cpool.tile([128, 4, E], BF16, tag="wv", name="wv_t")
            mk_t = cpool.tile([128, MKW], BF16, tag="mk", name="mk_t")
            ow_t = cpool.tile([128, 4, E], BF16, tag="ow", name="ow_t")
            nc.scalar.dma_start(mk_t[127:128, MKW - 8 : MKW],
                                wq_t[127:128, 3, E - 8 : E])
            nc.scalar.dma_start(mk_t[:], mk[:])
            nc.scalar.dma_start(wv_t[127:128, 3, E - 8 : E],
                                wq_t[127:128, 3, E - 8 : E])
            nc.scalar.dma_start(wv_t[:], wv[:].rearrange("(c p) e -> p c e", p=128))
            nc.sync.dma_start(ow_t[127:128, 3, E - 8 : E],
                              wk_t[127:128, 3, E - 8 : E])
            nc.sync.dma_start(ow_t[:], ow[:].rearrange("(c p) e -> p c e", p=128))

            qb = bb_t[:, 0:4]
            kb = bb_t[:, 4:8]
            obT = bb_t[:, 8:12]
            vb3 = bb_t[:, 12:524].rearrange("p (h d) -> p h d", d=HD)

            # small constants
            dum_t = cpool.tile([128, E], BF16, tag="dum", name="dum_t")
            nc.vector.memset(dum_t[:], 0.0)
            ones_t = cpool.tile([1, 64], BF16, tag="ones", name="ones_t")
            nc.gpsimd.memset(ones_t[:], 1.0)

            # ACT table warmup: force the ln/exp table load during the DMA
            # front instead of at the first real exp.
            actw = spool.tile([1, 8], FP32, tag="actw", name="actw")
            nc.vector.memset(actw[:], 1.0)
            nc.scalar.activation(actw[0:1, 0:4], actw[0:1, 4:8], AF.Ln)
            nc.scalar.activation(actw[0:1, 4:8], actw[0:1, 0:4], AF.Exp)

            # HAM warmup: dummy matmuls with no DMA deps run while the
            # input DMAs stream in, so the PE clock gate is at 8/8 when the
            # first projection matmul issues.
            for i in range(5):
                psd = ps.tile([HD + 1, CH], FP32, name=f"dum{i}", tag="psO")
                nc.tensor.matmul(
                    psd[:, 0:CH], dum_t[:, 0 : HD + 1], dum_t[:, 0:CH],
                    start=True, stop=True,
                )

            def keepalive(p):
                # Accumulate 0 into pair p's open psO group: a real MATMUL
                # (not reorderable like LDWEIGHTS) that marks the PE busy in
                # HAM's activity window, so sparse stretches don't drop the
                # clock to 4/8. Only legal while psO(p) is mid-accumulation.
                nc.tensor.matmul(
                    pso_tiles[p][0][:, 0:64], dum_t[:, 0 : HD + 1],
                    dum_t[:, 0:64], start=False, stop=False,
                    skip_group_check=True,
                )

            qT = [None] * 4
            kT = [None] * 4
            v_t = [None] * NT
            vT = [None] * 4
            att_tiles = {}
            pso_tiles = {}

            def emit_qproj(p):
                psq = ps.tile([128, CH], FP32, tag="pc", name=f"psq{p}")
                for kk in range(4):
                    nc.tensor.matmul(
                        psq[:],
                        wq_t[:, kk, 128 * p : 128 * p + 128],
                        xt_t[:, kk, 128 : 128 + CH],
                        start=(kk == 0), stop=(kk == 3),
                    )
                q = ppool.tile([128, CH], BF16, tag=f"qT{p}", name=f"qT{p}")
                nc.vector.tensor_scalar_add(q[:], psq[:], qb[:, p : p + 1])
                qT[p] = q

            def emit_kproj(p):
                kT[p] = ppool.tile([128, LK], BF16, tag=f"kT{p}", name=f"kT{p}")
                for h, (a, b) in enumerate(((0, 512), (512, 768))):
                    psk = ps.tile([128, b - a], FP32, tag="pc", name=f"psk{p}_{h}")
                    for kk in range(4):
                        nc.tensor.matmul(
                            psk[:],
                            wk_t[:, kk, 128 * p : 128 * p + 128],
                            xt_t[:, kk, a:b],
                            start=(kk == 0), stop=(kk == 3),
                        )
                    nc.vector.tensor_scalar_add(
                        kT[p][:, a:b], psk[:], kb[:, p : p + 1]
                    )

            def emit_vproj(m):
                psv = ps.tile([128, E], FP32, tag="pc", name=f"psv{m}")
                for kk in range(4):
                    nc.tensor.matmul(
                        psv[:],
                        xt_t[:, kk, 128 * m : 128 * m + 128],
                        wv_t[:, kk, :],
                        start=(kk == 0), stop=(kk == 3),
                    )
                v = ppool.tile([128, H * (HD + 1)], BF16, tag=f"v{m}", name=f"v{m}")
                v3 = v[:].rearrange("p (h d) -> p h d", d=HD + 1)
                psv3 = psv[:].rearrange("p (h d) -> p h d", d=HD)
                nc.vector.tensor_tensor(v3[:, :, 0:HD], psv3, vb3, op=AluOpType.add)
                nc.gpsimd.memset(v3[:, :, HD : HD + 1], 1.0)
                v_t[m] = v

            def emit_qk_chunk(p, c):
                """QK chunk c (512 cols of the concat layout) for both halves:
                matmuls into a PSUM chunk, exp on ACT -> att bf16, mask mul."""
                if c == 0:
                    attA = apool.tile([128, WSUM], BF16, name=f"attA{p}", tag="attA")
                    attB = apool.tile([128, WSUM], BF16, name=f"attB{p}", tag="attB")
                    att_tiles[p] = (attA, attB)
                for hh in range(2):
                    att = att_tiles[p][hh]
                    pcx = ps.tile([128, 512], FP32, tag="pc", name=f"pc{p}_{c}_{hh}")
                    r = slice(64 * hh, 64 * hh + 64)
                    for (t, q0, w, d0) in CHUNK_MMS[c]:
                        nc.tensor.matmul(
                            pcx[:, d0 : d0 + w],
                            kT[p][r, 128 * t : 128 * t + 128],
                            qT[p][r, q0 : q0 + w],
                            start=True, stop=True,
                        )
                    nc.scalar.activation(
                        att[:, 512 * c : 512 * c + 512], pcx[:], AF.Exp
                    )
                    # mask: mk cols 0:512 hold 0xFFFF/0 bit patterns -> c0
                    # runs as a uint32 bitwise AND on DVE (bitwise is
                    # DVE-only, 32-bit only; half the free size, ~1.8x
                    # cheaper than fp multiply). Chunks 1-2 hold fp 1.0/0.0
                    # (cols 512:1536) and multiply on GpSimd — except for
                    # pair 3, where the slow GpSimd op would sit on the
                    # kernel's tail: it uses a second bit-format copy
                    # (cols 1536:2560) and DVE ANDs.
                    sl = att[:, 512 * c : 512 * c + 512]
                    if c == 0 or p == 3:
                        mo = 512 * c if c == 0 else 1024 + 512 * c
                        slu = sl.bitcast(U32)
                        nc.vector.tensor_tensor(
                            slu, slu, mk_t[:, mo : mo + 512].bitcast(U32),
                            op=AluOpType.bitwise_and,
                        )
                    else:
                        nc.gpsimd.tensor_mul(
                            sl, sl, mk_t[:, 512 * c : 512 * c + 512]
                        )

            def emit_av(p, t):
                """AV matmuls for key tile t, both heads of pair p."""
                if t == 0:
                    pso_tiles[p] = (
                        ps.tile([HD + 1, CH], FP32, name=f"pso{2 * p}", tag="psO"),
                        ps.tile([HD + 1, CH], FP32, name=f"pso{2 * p + 1}", tag="psO"),
                    )
                w, qs, off = W_T[t], QS_T[t], OFF_T[t]
                for hh in range(2):
                    head = 2 * p + hh
                    nc.tensor.matmul(
                        pso_tiles[p][hh][:, qs : qs + w],
                        v_t[t][:, (HD + 1) * head : (HD + 1) * head + HD + 1],
                        att_tiles[p][hh][:, off : off + w],
                        start=(t == 0), stop=(t == NT - 1),
                        skip_group_check=True,
                    )

            def emit_norm(p):
                """Denominator -> reciprocal -> normalized values^T (bf16)."""
                psos = pso_tiles[p]
                # p<3: pc ring (interleaves with QK chunks). p==3: psO ring
                # (pair 2's slots are free by now) so all four psf partials
                # can hold the pc ring at the tail without deadlock.
                rbc = ps.tile([128, CH], FP32, tag="pc" if p < 3 else "psO",
                              name=f"rbc{p}")
                dens = []
                for hh in range(2):
                    den = spool.tile([1, CH], BF16, tag="den", name=f"den{2*p+hh}")
                    nc.vector.tensor_copy(den[:], psos[hh][HD : HD + 1, :])
                    dens.append(den)
                for hh in range(2):
                    nc.tensor.matmul(
                        rbc[64 * hh : 64 * hh + 64, :], ones_t[:], dens[hh][:],
                        start=True, stop=True,
                    )
                # recip = exp(-ln(den)) on ACT (custom-DVE reciprocal fails
                # codegen in this walrus build; DVE Newton costs ~3.5us/pair)
                lnv = spool.tile([128, CH], FP32, tag="lnv", name=f"lnv{p}")
                nc.scalar.activation(lnv[:], rbc[:], AF.Ln)
                rbs = spool.tile([128, CH], FP32, tag="rbs", name=f"rbs{p}")
                nc.scalar.activation(rbs[:], lnv[:], AF.Exp, scale=-1.0)
                vtn = ppool.tile([128, CH], BF16, tag=f"vT{p}", name=f"vT{p}")
                for hh in range(2):
                    nc.vector.tensor_mul(
                        vtn[64 * hh : 64 * hh + 64, :],
                        psos[hh][0:HD, :],
                        rbs[64 * hh : 64 * hh + 64, :],
                    )
                vT[p] = vtn

            # ================= emission schedule =================
            emit_qproj(0); emit_kproj(0)
            for c in range(3):
                emit_qk_chunk(0, c)
            emit_qproj(1); emit_kproj(1)
            emit_vproj(0); emit_vproj(1)
            for c in range(3):
                emit_qk_chunk(1, c)
            emit_av(0, 0); emit_av(0, 1)
            emit_qproj(2); emit_kproj(2)
            emit_vproj(2)
            emit_av(0, 2)
            emit_vproj(3)
            emit_av(0, 3)
            for c in range(3):
                emit_qk_chunk(2, c)
            emit_av(1, 0); emit_av(1, 1)
            emit_qproj(3); emit_kproj(3)
            emit_vproj(4)
            emit_av(0, 4)
            emit_av(1, 2)
            emit_vproj(5)
            emit_av(0, 5)          # psO(0) complete
            emit_norm(0)
            emit_av(1, 3)
            emit_qk_chunk(3, 0)
            emit_av(1, 4); emit_av(1, 5)   # psO(1) complete
            emit_norm(1)
            emit_qk_chunk(3, 1)
            emit_av(2, 0); emit_av(2, 1)
            emit_qk_chunk(3, 2)
            emit_av(2, 2); emit_av(2, 3)
            keepalive(2)
            emit_av(2, 4); emit_av(2, 5)
            emit_norm(2)
            emit_av(3, 0); emit_av(3, 1)
            keepalive(3)
            emit_av(3, 2); emit_av(3, 3)
            keepalive(3)
            emit_av(3, 4); emit_av(3, 5)

            # ---- output projection (transposed: psfT[e, q]) ----
            # all four m-blocks accumulate p=0..2 while pair 3 normalizes;
            # only the p=3 matmul + bias-evict + store remain after norm(3).
            psf = [None] * 4

            def emit_psf_mms(m, ps_, pe_):
                if psf[m] is None:
                    psf[m] = ps.tile([128, CH], FP32, tag="pc", name=f"psf{m}")
                for p in range(ps_, pe_):
                    nc.tensor.matmul(
                        psf[m][:],
                        ow_t[:, p, 128 * m : 128 * m + 128],
                        vT[p][:],
                        start=(p == 0), stop=(p == 3),
                    )

            def emit_psf_out(m):
                fin = fpool.tile([128, E], BF16, tag="fin", name=f"fin{m}")
                if m % 2 == 0:
                    nc.vector.tensor_scalar_add(fin[:], psf[m][:], obT[:, m : m + 1])
                else:
                    nc.scalar.activation(fin[:], psf[m][:], AF.Identity,
                                         bias=obT[:, m : m + 1])
                dq = nc.sync if m % 2 == 0 else nc.scalar
                dq.dma_start(out[128 * m : 128 * m + 128, :], fin[:])

            for m in range(4):
                emit_psf_mms(m, 0, 3)
            emit_norm(3)
            for m in range(4):
                emit_psf_mms(m, 3, 4)
                emit_psf_out(m)

    return nc


_NC_CACHE = None


def _get_program():
    global _NC_CACHE
    if _NC_CACHE is None:
        _NC_CACHE = _build_program()
    return _NC_CACHE


def _make_in_maps(x, padding_mask, qkv_w, qkv_b, o_w, o_b):
    x = np.asarray(x, np.float32)
    pm = np.asarray(padding_mask)
    qkv_w = np.asarray(qkv_w, np.float32)
    qkv_b = np.asarray(qkv_b, np.float32)
    o_w = np.asarray(o_w, np.float32)
    o_b = np.asarray(o_b, np.float32)

    scale = np.float32(1.0 / np.sqrt(HD))
    # reference splits per-head: head h uses qkv rows [192h,192h+64) (q),
    # +64 (k), +128 (v)
    idx_q = np.concatenate([np.arange(3 * HD * h, 3 * HD * h + HD) for h in range(H)])
    idx_k = idx_q + HD
    idx_v = idx_q + 2 * HD

    wq = np.ascontiguousarray((qkv_w[idx_q] * scale).T.astype(BF))  # [IN, E]
    wk = np.ascontiguousarray(qkv_w[idx_k].T.astype(BF))
    wv = np.ascontiguousarray(qkv_w[idx_v].T.astype(BF))
    ow = np.ascontiguousarray(o_w.T.astype(BF))                     # [f, e]

    bigb = np.zeros((128, 524), np.float32)
    bigb[:, 0:4] = (qkv_b[idx_q] * scale).reshape(4, 128).T
    bigb[:, 4:8] = qkv_b[idx_k].reshape(4, 128).T
    bigb[:, 8:12] = o_b.reshape(4, 128).T
    bigb[:, 12:524] = np.broadcast_to(qkv_b[idx_v][None, :], (128, E))

    xb = x.astype(BF)

    j = np.arange(128)[:, None]
    in_maps = []
    for c in range(8):
        b, ch = divmod(c, 4)
        s0 = CH * ch
        lo, hi = max(0, s0 - HW), min(S, s0 + CH + HW)
        xpad = np.zeros((LK, IN_DIM), BF)
        xpad[lo - (s0 - HW) : hi - (s0 - HW)] = xb[b, lo:hi]
        xt = np.ascontiguousarray(xpad.T)                     # [IN, LK] bf16

        mask = np.zeros((128, WSUM), np.float32)
        for t in range(NT):
            w, qs, off = W_T[t], QS_T[t], OFF_T[t]
            lk = 128 * t + j                                  # [128,1] local key
            q = qs + np.arange(w)[None, :]                    # [1,w] local query
            band = (q <= lk) & (lk <= q + 2 * HW)
            gk = s0 - HW + lk                                 # global key index
            valid = (gk >= 0) & (gk < S)
            pmk = pm[b, np.clip(gk, 0, S - 1)] != 0
            mask[:, off : off + w] = (band & valid & pmk).astype(np.float32)

        # cols 0:512 = chunk0 as 0xFFFF/0 bit patterns (DVE AND, all pairs);
        # cols 512:1536 = chunks 1-2 as fp 1.0/0.0 (GpSimd mult, pairs 0-2);
        # cols 1536:2560 = chunks 1-2 as bit patterns (DVE AND, pair 3)
        mkb = np.zeros((128, MKW), BF)
        bits = np.where(mask != 0, np.uint16(0xFFFF), np.uint16(0)).view(BF)
        mkb[:, 0:512] = bits[:, 0:512]
        mkb[:, 512:1536] = mask[:, 512:1536].astype(BF)
        mkb[:, 1536:2560] = bits[:, 512:1536]
        in_maps.append(
            {"xt": xt, "wq": wq, "wk": wk, "wv": wv, "ow": ow,
             "mk": mkb, "bigb": bigb}
        )
    return in_maps


def _run(x, padding_mask, qkv_w, qkv_b, o_w, o_b, trace=False, tmpdir=None):
    nc = _get_program()
    in_maps = _make_in_maps(x, padding_mask, qkv_w, qkv_b, o_w, o_b)
    res = run_bass_kernel_spmd(
        nc, in_maps, core_ids=list(range(8)), trace=trace, tmpdir=tmpdir
    )
    o = np.empty((B, S, E), np.float32)
    for c in range(8):
        b, ch = divmod(c, 4)
        o[b, CH * ch : CH * ch + CH, :] = (
            res.results[c]["out"].astype(np.float32).T
        )
    # fully-masked query rows: att = 0 -> output is exactly the bias
    pm = np.asarray(padding_mask)
    if (pm == 0).any():
        o[pm == 0] = np.asarray(o_b, np.float32)
    return o, res


def kernel(x, padding_mask, qkv_w, qkv_b, o_w, o_b, window_size, num_heads):
    assert int(window_size) == WS and int(num_heads) == H
    assert tuple(np.asarray(x).shape) == (B, S, IN_DIM)
    o, _ = _run(x, padding_mask, qkv_w, qkv_b, o_w, o_b)
    return o
